# revision 32
# baseline (speedup 1.0000x reference)
"""Trainium2 Bass kernel for nn_NeuroScribe: CNN feature extractor + DMP integrator.

Strategy (per core, 512 samples, pure data-parallel across 8 cores):
  - Host folds L_w into fc_w (only 7 FC outputs needed: goal, w[5], tau) and
    builds a pooled-parity im2col for conv1 in fp16: each moving column
    (56 rows = 11-pos window x 5ci + bias) computes 64 channels at TWO raw
    positions r and r+4 that live in different pool1 windows, so the matmul
    output partition dim is (pool1-window parity, channel) = 128 with every
    output useful (98304 moving columns total -- the minimum).
  - conv1: 192 matmuls K=56 N=512 (1-bank PSUM tiles, 4-deep pools) with
    relu+pool4 fused into the eviction: ACT relu-evicts the po tiles, DVE
    scalar_tensor_tensor folds in the pe tiles, GPSIMD does the last fold
    -> h1 [128=(par,ci), 50, 512] fp16 (cols 0/49 zero pads).
  - conv2: because h1's partition dim packs two CONSECUTIVE pooled positions,
    the 5-tap conv contracts as 3 accumulating feeds of K=128/128/64
    (tap pairs {0,1}{2,3}{4} for even outputs, {1,2}{3,4}{0} for odd) --
    147456 moving columns, the K<=128 minimum. Eviction fuses bias+relu and
    both pool2 folds -> h2p [128, 24, 512] fp16.
  - fc: 24 accumulating matmuls with w7 stationary -> [7, 512] PSUM, then
    4 PE identity-transposes -> g7 [128 samples, 4, 7].
  - DMP: closed form. B_Z = A_Z/4 => critically damped: the 2x2 transition is
    lam*I + N with N nilpotent. x_t = d^t (geometric).
        y_t = lam^t y0 + t lam^(t-1) q1 + u S_t,  q1 = u(12.5 y0 + u)
        C_{t+1} = lam C_t + beta_t ; S_{t+1} = lam S_t + C_t
        beta_t = u (156.25 goal + fx_t)
    All four 128-sample chunks run in ONE tensor_tensor_scan per recurrence
    (a 0 in data0 at each chunk's first column resets the scan state);
    per-sample scalar prep is batched into [128, 4] ops; psi squares run on
    DVE/Pool so ACT only does the exps; g7-independent prep is emitted
    before the conv phase to fill idle engine time.
  - All DMA writes span multiple-of-8 partition counts: partial-partition
    DMA writes (e.g. 71 rows) hit a ~25 GB/s descriptor slow path -- this
    was the single biggest win (2x) over the first working version.
"""
import os
import numpy as np

import concourse.bass as bass
import concourse.bacc as bacc
import concourse.mybir as mybir
from concourse import tile
from concourse import bass_utils

f32 = mybir.dt.float32
f16 = mybir.dt.float16
i32 = mybir.dt.int32
AF = mybir.ActivationFunctionType
ALU = mybir.AluOpType

N_CORES = 8
B = 4096
BC = B // N_CORES          # 512 samples per core
T = 101
NT = 100                   # scan steps
DT = 0.01
N_RBF = 5
_C = np.exp(-np.linspace(0.0, 1.0, N_RBF)).astype(np.float32)
_SIG2 = ((N_RBF ** 1.5) / _C).astype(np.float32)

L1 = 384                   # conv1 raw positions
Q1 = 96                    # pooled positions after pool1
Q2 = 24                    # pooled positions after pool2
K1 = 56                    # conv1 contraction: 11-pos window x 5ci + bias
NQ = 48                    # conv1 emission groups (pool1-window pairs)
NCOL = 192                 # im2col columns (4 per group: raw 8Q+j, j<4)
NCOL_STRIP = 32            # im2col columns per DMA strip
N_STRIPS = NCOL // NCOL_STRIP


def _emit_dmp_prep(nc, dp):
    """g7-independent DMP constants; emitted before the conv phase so the
    scheduler fills idle engine time with them."""
    p = {}
    ones = p["ones"] = dp.tile([128, T], f32, name="ones")
    nc.vector.memset(ones[:], 1.0)
    tio = dp.tile([128, T], i32, name="tio")
    nc.gpsimd.iota(tio[:], [[1, T]], base=0, channel_multiplier=0)
    tful = p["tful"] = dp.tile([128, T], f32, name="tful")
    nc.vector.tensor_copy(tful[:], tio[:])
    tful4 = p["tful4"] = dp.tile([128, 4, T], f32, name="tful4")
    for c in range(4):
        nc.gpsimd.tensor_copy(tful4[:, c, :], tful[:])
    e0 = p["e0"] = dp.tile([128, 4, T], f32, name="e0")
    nc.vector.memset(e0.rearrange("p a b -> p (a b)"), 0.0)
    nc.vector.memset(e0[:, :, 0:1], 1.0)
    return p


def _emit_dmp(nc, dp, prep, g7, y0t, outd):
    ones = prep["ones"]
    tful4 = prep["tful4"]
    e0 = prep["e0"]

    xs = dp.tile([128, 4, T], f32)
    lamt = dp.tile([128, 4, T], f32)
    lamf_all = dp.tile([128, 4, T], f32)
    dgf_all = dp.tile([128, 4, T], f32)
    Cs = dp.tile([128, 4, T], f32)
    Cs_sh = dp.tile([128, 4, T], f32)
    Ss = dp.tile([128, 4, T], f32)
    beta = dp.tile([128, 4, T], f32)
    num = dp.tile([128, 4, T], f32)
    den = dp.tile([128, 4, T], f32)
    fx2 = dp.tile([128, 4, T], f32)
    yout = dp.tile([128, 4, T], f32)
    psi = [dp.tile([128, 4, T], f32, name=f"psi{j}")
           for j in range(N_RBF)]

    def R2(t):
        return t.rearrange("p a b -> p (a b)")

    # per-sample scalars, batched over the 4 sample chunks: [128, 4] ops
    sc = dp.tile([128, 12, 4], f32)
    tau4 = R2(g7[:, :, 6:7])
    goal4 = R2(g7[:, :, 0:1])
    u4, lam4, dg4, kgy4 = (sc[:, i, :] for i in range(4))
    q1l4, bsc4, bct4, t04 = (sc[:, i, :] for i in range(4, 8))
    t14, rl4, lndg4, lnlam4 = (sc[:, i, :] for i in range(8, 12))
    nc.vector.tensor_scalar_mul(u4, tau4, DT)
    nc.vector.tensor_scalar(lam4, tau4, -0.125, 1.0, ALU.mult, ALU.add)
    nc.vector.tensor_scalar(dg4, tau4, -0.01, 1.0, ALU.mult, ALU.add)
    nc.vector.tensor_sub(kgy4, goal4, y0t[:, :])
    nc.vector.scalar_tensor_tensor(t04, y0t[:, :], 12.5, u4,
                                   ALU.mult, ALU.add)
    nc.vector.tensor_mul(t14, u4, t04)
    nc.vector.reciprocal(rl4, lam4)
    nc.vector.tensor_mul(q1l4, t14, rl4)
    nc.vector.tensor_mul(bsc4, u4, kgy4)
    nc.vector.tensor_mul(t04, tau4, goal4)
    nc.vector.tensor_scalar_mul(bct4, t04, 1.5625)
    # xs = dg^t, lamt = lam^t, and later Cs/Ss: each is ONE scan over
    # [128, 4*T] -- a 0 in data0 at every chunk's col 0 resets the scan
    # state (state = 0*state + data1), giving 4 independent scans per op
    nc.vector.memset(dgf_all[:, :, 0:1], 0.0)
    nc.vector.memset(lamf_all[:, :, 0:1], 0.0)
    for c in range(4):
        beng = nc.vector if c < 2 else nc.gpsimd
        beng.tensor_scalar_mul(dgf_all[:, c, 1:T], ones[:, 0:NT],
                               dg4[:, c:c + 1])
        beng.tensor_scalar_mul(lamf_all[:, c, 1:T], ones[:, 0:NT],
                               lam4[:, c:c + 1])
    nc.vector.tensor_tensor_scan(
        R2(xs), R2(dgf_all), R2(e0), 0.0, ALU.mult, ALU.add)
    nc.vector.tensor_tensor_scan(
        R2(lamt), R2(lamf_all), R2(e0), 0.0, ALU.mult, ALU.add)
    nc.vector.memset(Cs_sh[:, :, 0:1], 0.0)

    for j in range(N_RBF):
        nc.vector.tensor_scalar(R2(psi[j]), R2(xs), float(-_C[j]), 0.0,
                                ALU.add, ALU.bypass)
        meng = nc.vector if j % 2 == 0 else nc.gpsimd
        meng.tensor_mul(R2(psi[j]), R2(psi[j]), R2(psi[j]))
        nc.scalar.activation(R2(psi[j]), R2(psi[j]), AF.Exp,
                             scale=float(-0.5 / _SIG2[j]))
    nc.gpsimd.tensor_add(R2(den), R2(psi[0]), R2(psi[1]))
    nc.gpsimd.tensor_add(R2(fx2), R2(psi[2]), R2(psi[3]))
    nc.gpsimd.tensor_add(R2(den), R2(den), R2(fx2))
    nc.gpsimd.tensor_add(R2(den), R2(den), R2(psi[4]))
    nc.vector.reciprocal(R2(den), R2(den))

    for c in range(4):
        ncol = num[:, c, :]
        if c < 2:
            nc.vector.tensor_scalar_mul(ncol, psi[0][:, c, :],
                                        g7[:, c, 1:2])
            for j in range(1, N_RBF):
                nc.vector.scalar_tensor_tensor(
                    ncol, psi[j][:, c, :], g7[:, c, 1 + j:2 + j],
                    ncol, ALU.mult, ALU.add)
        else:
            tmp = Cs[:, c, :]
            nc.gpsimd.tensor_scalar_mul(ncol, psi[0][:, c, :],
                                        g7[:, c, 1:2])
            for j in range(1, N_RBF):
                nc.gpsimd.tensor_scalar_mul(tmp, psi[j][:, c, :],
                                            g7[:, c, 1 + j:2 + j])
                nc.gpsimd.tensor_add(ncol, ncol, tmp)
    nc.vector.tensor_mul(R2(fx2), R2(num), R2(den))
    nc.vector.tensor_mul(R2(fx2), R2(fx2), R2(xs))

    nc.vector.memset(beta[:, :, 0:1], 0.0)
    for c in range(4):
        nc.vector.tensor_scalar(beta[:, c, 1:T], fx2[:, c, 0:NT],
                                bsc4[:, c:c + 1], bct4[:, c:c + 1],
                                ALU.mult, ALU.add)
    nc.vector.tensor_tensor_scan(
        R2(Cs), R2(lamf_all), R2(beta), 0.0, ALU.mult, ALU.add)
    nc.gpsimd.tensor_copy(Cs_sh[:, :, 1:T], Cs[:, :, 0:NT])
    nc.vector.tensor_tensor_scan(
        R2(Ss), R2(lamf_all), R2(Cs_sh), 0.0, ALU.mult, ALU.add)
    # y = lamt*y0 + (t*lamt)*q1 + u*S
    nc.gpsimd.tensor_mul(R2(num), R2(lamt), R2(tful4))
    for c in range(4):
        nc.gpsimd.tensor_scalar_mul(den[:, c, :], lamt[:, c, :],
                                    y0t[:, c:c + 1])
        nc.vector.scalar_tensor_tensor(
            num[:, c, :], num[:, c, :], q1l4[:, c:c + 1], den[:, c, :],
            ALU.mult, ALU.add)
        nc.vector.scalar_tensor_tensor(
            yout[:, c, :], Ss[:, c, :], u4[:, c:c + 1], num[:, c, :],
            ALU.mult, ALU.add)

    nc.sync.dma_start(outd[:], yout[:])


def build_program(weights, repeat=1):
    nc = bacc.Bacc(None, target_bir_lowering=False, debug=True)

    x1d = nc.dram_tensor("x1", [K1, NCOL, BC], f16, kind="ExternalInput")
    y0d = nc.dram_tensor("y0c", [128, 4], f32, kind="ExternalInput")
    outd = nc.dram_tensor("out", [128, 4, T], f32, kind="ExternalOutput")

    w1d = nc.inline_tensor(weights["W1q"], "W1q")       # [K1, 128] f16
    w2d = nc.inline_tensor(weights["W2S"], "W2S")       # [128, 6, 128] f16
    w7d = nc.inline_tensor(weights["W7t"], "W7t")       # [128, 24, 7] f16
    b2d = nc.inline_tensor(weights["b2c"], "b2c")       # [128, 1] f32
    b7d = nc.inline_tensor(weights["b7rep"], "b7rep")   # [128, 7] f32
    id8d = nc.inline_tensor(np.eye(8, dtype=np.float32), "id8")

    with tile.TileContext(nc) as tc:
      for _rep in range(repeat):
        with tc.tile_pool(name="const", bufs=1) as cp:
            w1t = cp.tile([K1, 128], f16)
            nc.sync.dma_start(w1t[:], w1d[:])
            w2t = cp.tile([128, 6, 128], f16)
            nc.sync.dma_start(w2t[:], w2d[:])
            w7t = cp.tile([128, 24, 7], f16)
            nc.sync.dma_start(w7t[:], w7d[:])
            b2t = cp.tile([128, 1], f32)
            nc.sync.dma_start(b2t[:], b2d[:])
            b7t = cp.tile([128, 7], f32)
            nc.sync.dma_start(b7t[:], b7d[:])
            y0t = cp.tile([128, 4], f32)
            nc.sync.dma_start(y0t[:], y0d[:])

            # h1: [128=(pool-parity, ci), 50, 512] fp16; col t = pooled pair
            # (2(t-1), 2(t-1)+1); cols 0 and 49 zero pads
            h1 = cp.tile([128, NQ + 2, BC], f16)
            nc.vector.memset(h1[:, 0:1, :], 0.0)
            nc.vector.memset(h1[:, NQ + 1:NQ + 2, :], 0.0)
            # h2p: [128=co2, 24 pooled2, 512] fp16
            h2p = cp.tile([128, Q2, BC], f16, name="h2p")
            g7 = cp.tile([128, 4, 7], f32, name="g7")

            dmp_prep = _emit_dmp_prep(nc, cp)

            # ---------- conv1 + conv2, interleaved, shared psum pools -------
            with tc.tile_pool(name="x1p", bufs=2) as xp, \
                 tc.tile_pool(name="psA", bufs=4, space="PSUM") as psA, \
                 tc.tile_pool(name="psB", bufs=4, space="PSUM") as psB, \
                 tc.tile_pool(name="stg", bufs=3) as stp:

                x1tiles = []

                def conv1_group(Q):
                    # group Q: 4 im2col cols -> h1 tile col Q+1
                    s, g = divmod(Q, NCOL_STRIP // 4)
                    if g == 0:
                        x1t = xp.tile([K1, NCOL_STRIP, BC], f16, tag="x1t",
                                      name=f"x1t{s}")
                        half = (NCOL_STRIP // 2) * BC
                        flat = x1t.rearrange("p a b -> p (a b)")
                        src = x1d[:, s * NCOL_STRIP:(s + 1) * NCOL_STRIP, :]
                        srcf = src.rearrange("p a b -> p (a b)")
                        if s == 0:
                            qh = half // 2
                            for hh in range(4):
                                eng = nc.sync if hh % 2 == 0 else nc.scalar
                                eng.dma_start(
                                    flat[:, hh * qh:(hh + 1) * qh],
                                    srcf[:, hh * qh:(hh + 1) * qh])
                        else:
                            nc.sync.dma_start(flat[:, 0:half],
                                              srcf[:, 0:half])
                            nc.sync.dma_start(flat[:, half:2 * half],
                                              srcf[:, half:2 * half])
                        x1tiles.append(x1t)
                    x1t = x1tiles[s]
                    c0 = g * 4
                    pe_t = [psA.tile([128, BC], f32, tag="psE",
                                     name=f"c1e{Q}_{jj}") for jj in range(2)]
                    po_t = [psB.tile([128, BC], f32, tag="psO",
                                     name=f"c1o{Q}_{jj}") for jj in range(2)]
                    for jj in range(2):
                        nc.tensor.matmul(po_t[jj][:], w1t[:, :],
                                         x1t[:, c0 + 2 + jj, :],
                                         start=True, stop=True)
                    for jj in range(2):
                        nc.tensor.matmul(pe_t[jj][:], w1t[:, :],
                                         x1t[:, c0 + jj, :],
                                         start=True, stop=True)
                    o_s = stp.tile([128, 2, BC], f16, tag="o_s",
                                   name=f"c1os{Q}")
                    l1t = stp.tile([128, 2, BC], f16, tag="l1t",
                                   name=f"c1l{Q}")
                    for jj in range(2):
                        nc.scalar.activation(o_s[:, jj, :], po_t[jj][:],
                                             AF.Relu)
                        nc.vector.scalar_tensor_tensor(
                            l1t[:, jj, :], pe_t[jj][:], 0.0, o_s[:, jj, :],
                            ALU.max, ALU.add)
                    nc.gpsimd.tensor_add(
                        h1[:, Q + 1, :], l1t[:, 0, :], l1t[:, 1, :])

                def conv2_group(P):
                    # outputs q in {4P..4P+3} -> h2p col P
                    t0 = 2 * P  # h1 tile col base
                    pe_t = [psA.tile([128, BC], f32, tag="psE",
                                     name=f"c2e{P}_{jj}") for jj in range(2)]
                    po_t = [psB.tile([128, BC], f32, tag="psO",
                                     name=f"c2o{P}_{jj}") for jj in range(2)]
                    for jj in range(2):
                        nc.tensor.matmul(po_t[jj][:], w2t[:, 3, :],
                                         h1[:, t0 + 1 + jj, :],
                                         start=True, stop=False)
                        nc.tensor.matmul(po_t[jj][:], w2t[:, 4, :],
                                         h1[:, t0 + 2 + jj, :],
                                         start=False, stop=False)
                        nc.tensor.matmul(po_t[jj][:], w2t[64:128, 5, :],
                                         h1[64:128, t0 + jj, :],
                                         start=False, stop=True)
                    for jj in range(2):
                        nc.tensor.matmul(pe_t[jj][:], w2t[:, 0, :],
                                         h1[:, t0 + jj, :],
                                         start=True, stop=False)
                        nc.tensor.matmul(pe_t[jj][:], w2t[:, 1, :],
                                         h1[:, t0 + 1 + jj, :],
                                         start=False, stop=False)
                        nc.tensor.matmul(pe_t[jj][:], w2t[0:64, 2, :],
                                         h1[0:64, t0 + 2 + jj, :],
                                         start=False, stop=True)
                    o_s = stp.tile([128, 2, BC], f16, tag="o_s",
                                   name=f"c2os{P}")
                    e_s = stp.tile([128, 2, BC], f16, tag="l1t",
                                   name=f"c2es{P}")
                    for jj in range(2):
                        nc.scalar.activation(o_s[:, jj, :], po_t[jj][:],
                                             AF.Relu, bias=b2t[:, 0:1])
                        nc.vector.tensor_scalar(e_s[:, jj, :], pe_t[jj][:],
                                                b2t[:, 0:1], 0.0,
                                                ALU.add, ALU.max)
                    l2t = stp.tile([128, 2, BC], f16, tag="l2t",
                                   name=f"c2l{P}")
                    nc.gpsimd.tensor_add(l2t[:], e_s[:], o_s[:])
                    nc.gpsimd.tensor_add(
                        h2p[:, P, :], l2t[:, 0, :], l2t[:, 1, :])

                # conv2 group P reads h1 tile cols 2P..2P+3 -> needs conv1
                # groups through Q = 2P+2 (tile col 2P+3) evicted first.
                emitted = 0
                for Q in range(NQ):
                    while emitted < Q2 and 2 * emitted + 2 <= Q - 1:
                        conv2_group(emitted)
                        emitted += 1
                    conv1_group(Q)
                    while emitted < Q2 and 2 * emitted + 2 <= Q:
                        conv2_group(emitted)
                        emitted += 1
                while emitted < Q2:
                    conv2_group(emitted)
                    emitted += 1

            # -------- fc: [7, 512] via 24 matmuls, PE-transpose to g7 -----
            with tc.tile_pool(name="psg", bufs=1, space="PSUM") as psg, \
                 tc.tile_pool(name="psh", bufs=2, space="PSUM") as psh, \
                 tc.tile_pool(name="fcs", bufs=1) as fcs:
                gq = psg.tile([8, BC], f32)
                for q4 in range(Q2):
                    nc.tensor.matmul(
                        gq[0:7, :], w7t[:, q4, :], h2p[:, q4, :],
                        start=(q4 == 0), stop=(q4 == Q2 - 1))
                gsb = fcs.tile([8, BC], f32)
                nc.vector.tensor_copy(gsb[0:7, :], gq[0:7, :])
                idn = fcs.tile([8, 8], f32)
                nc.sync.dma_start(idn[:], id8d[:])
                for ch in range(4):
                    gt = psh.tile([128, 7], f32, tag="gt")
                    nc.tensor.matmul(
                        gt[:], gsb[0:7, ch * 128:(ch + 1) * 128],
                        idn[0:7, 0:7], is_transpose=True)
                    nc.vector.tensor_add(g7[:, ch, :], gt[:], b7t[:])

            # ---------------- DMP closed form ----------------
            _emit_dmp(nc, cp, dmp_prep, g7, y0t, outd)

    nc.compile()
    return nc


# --------------------------------------------------------------------------
# host-side prep
# --------------------------------------------------------------------------

def prep_weights(conv1_w, conv1_b, conv2_w, conv2_b, fc_w, fc_b, L_w, L_b):
    # W1q: col c computes raw position r (even pool window), col 64+c
    # computes r+4 (odd window); rows = (window offset o, ci), o in 0..10.
    W1q = np.zeros((K1, 128), np.float32)
    wt = conv1_w.transpose(2, 1, 0).reshape(35, 64)   # rows (o, ci)
    W1q[0:35, 0:64] = wt
    W1q[20:55, 64:128] = wt
    W1q[55, 0:64] = conv1_b
    W1q[55, 64:128] = conv1_b
    # conv2 tap-pair stationaries; x0.25 folds the pool1 mean.
    # slots: 0/1/2 even outputs (taps {0,1},{2,3},{4-top64});
    #        3/4/5 odd outputs  (taps {1,2},{3,4},{0-bottom64})
    W2S = np.zeros((128, 6, 128), np.float32)
    tp = [conv2_w[:, :, k].T * 0.25 for k in range(5)]  # [ci, co]
    W2S[0:64, 0, :], W2S[64:128, 0, :] = tp[0], tp[1]
    W2S[0:64, 1, :], W2S[64:128, 1, :] = tp[2], tp[3]
    W2S[0:64, 2, :] = tp[4]
    W2S[0:64, 3, :], W2S[64:128, 3, :] = tp[1], tp[2]
    W2S[0:64, 4, :], W2S[64:128, 4, :] = tp[3], tp[4]
    W2S[64:128, 5, :] = tp[0]
    Wfc7 = np.concatenate(
        [fc_w[0:6].astype(np.float64),
         (L_w.astype(np.float64) @ fc_w.astype(np.float64))], axis=0)
    W7t = np.zeros((128, Q2, 7), np.float32)
    for j in range(7):
        W7t[:, :, j] = Wfc7[j].reshape(128, Q2) * 0.25
    b7 = np.concatenate(
        [fc_b[0:6].astype(np.float64),
         L_w.astype(np.float64) @ fc_b.astype(np.float64)
         + L_b.astype(np.float64)])
    return {
        "W1q": W1q.astype(np.float16),
        "W2S": W2S.astype(np.float16),
        "W7t": W7t.astype(np.float16),
        "b2c": conv2_b.reshape(128, 1).astype(np.float32),
        "b7rep": np.tile(b7.astype(np.float32)[None, :], (128, 1)),
    }


def prep_core_inputs(input_full, y0_full, core):
    base = core * BC
    inp = input_full[base:base + BC]                  # [512, 5, 384]
    inp_pad = np.zeros((BC, 5, L1 + 10), np.float32)
    inp_pad[:, :, 3:3 + L1] = inp                     # padded idx = r + o
    m = np.arange(NCOL)
    r_idx = (m // 4) * 8 + (m % 4)                    # raw position per col
    X1 = np.empty((K1, NCOL, BC), np.float16)
    for o in range(11):
        for ci in range(5):
            X1[o * 5 + ci] = inp_pad[:, ci, :][:, r_idx + o].T
    X1[55] = 1.0
    y0dev = y0_full[base:base + BC].astype(np.float32).reshape(4, 128).T
    return {"x1": X1, "y0c": np.ascontiguousarray(y0dev)}


_CACHE = {}
LAST_RESULTS = None


def kernel(input, y0, conv1_w, conv1_b, conv2_w, conv2_b, fc_w, fc_b, L_w, L_b):
    key = "nc"
    if key not in _CACHE:
        weights = prep_weights(conv1_w, conv1_b, conv2_w, conv2_b,
                               fc_w, fc_b, L_w, L_b)
        _CACHE[key] = build_program(
            weights, repeat=int(os.environ.get("KERNEL_REPEAT", "1")))
    nc = _CACHE[key]

    in_maps = [prep_core_inputs(input, y0, core) for core in range(N_CORES)]

    trace = bool(int(os.environ.get("KERNEL_TRACE", "0")))
    res = bass_utils.run_bass_kernel_spmd(
        nc, in_maps, core_ids=list(range(N_CORES)), trace=trace)
    global LAST_RESULTS
    LAST_RESULTS = res

    out = np.empty((B, T, 1), np.float32)
    for core in range(N_CORES):
        ydev = res.results[core]["out"].transpose(1, 0, 2).reshape(BC, T)
        out[core * BC:(core + 1) * BC, :, 0] = ydev
    return out
